# revision 5
# baseline (speedup 1.0000x reference)
"""Trainium2 Bass kernel for nn_AST_LSTM (GRU-based AST message passing).

Algorithm notes
---------------
The reference peels leaf edges of a random tree for 15 iterations; the
edge schedule (which edges fire when, and the compacted index remapping)
depends ONLY on E, so it is precomputed on the host. Per iteration the
device does, for each of 8 row-sharded cores:

    q = S_k @ h            (sparse mean-aggregate of gathered rows)
    G = [q | h] @ Wcat + b  with Wcat = [conv_w @ w_ih.T ; w_hh.T]
    r = sigmoid(G[:, :384]); z = sigmoid(G[:, 384:768])
    n = tanh(gi_n + b_ih_n + r * (gh_n + b_hh_n))
    h' = n + z * (h - n)

Rows are block-cyclically sharded (block=384) over 8 cores; each core
holds h transposed in SBUF as 10 window tiles [128, 3, 384] (feature
partition-major) so all matmuls run directly.  Message sources are
exchanged once per iteration with an AllGather of just the needed rows
(the "halo"); iteration 0 gathers from a full local copy of V.  All
device data is bf16 (validated: final rel err ~1e-3); PSUM accumulates
fp32; biases are injected into PSUM via K=1 ones-matmuls.
"""
import os
import sys
import numpy as np

sys.path.insert(0, "/opt/trn_rl_repo")
import ml_dtypes

N = 30000
D = 384
NC = 8
ITERS = int(os.environ.get("KERNEL_ITERS", "15"))
B = 384              # assignment block == window width
WPC = 10             # windows per core
LROWS = B * WPC      # 3840 local rows (padded)
NBLKS = (N + B - 1) // B
OOB = 1 << 20
BF16 = ml_dtypes.bfloat16

# ----------------------------------------------------------------------------
# host-side schedule
# ----------------------------------------------------------------------------

def _local_row(g):
    return ((g // B) // NC) * B + g % B


def _global_rows_of(c):
    out = np.full(LROWS, -1, dtype=np.int64)
    for w in range(WPC):
        blk = w * NC + c
        if blk >= NBLKS:
            continue
        g0 = blk * B
        n = min(B, N - g0)
        out[w * B: w * B + n] = np.arange(g0, g0 + n)
    return out


def _peel_schedule(E):
    src = np.asarray(E[0], dtype=np.int64)
    dst = np.asarray(E[1], dtype=np.int64)
    M = src.shape[0]
    active = np.ones(M, dtype=bool)
    iters = []
    for _ in range(ITERS):
        tgt_cnt = np.zeros(N, np.int64)
        np.add.at(tgt_cnt, dst, active.astype(np.int64))
        use = active & (tgt_cnt == 0)[src]
        ui = use.astype(np.int64)
        node_used = np.zeros(N, np.int64)
        np.maximum.at(node_used, src, ui)
        np.maximum.at(node_used, dst, ui)
        index_map = np.cumsum(node_used) - 1
        s_idx = index_map[src[use]]
        t_idx = index_map[dst[use]]
        cnt = np.zeros(N, np.int64)
        np.add.at(cnt, t_idx, 1)
        iters.append((s_idx, t_idx, cnt))
        active = active & ~use
    return iters


def build_schedule(E):
    """Static schedule: identical program structure for all cores, per-core
    index/matrix data (padded to union shapes)."""
    peel = _peel_schedule(E)
    its = []
    for k in range(ITERS):
        s_idx, t_idx, cnt = peel[k]
        it = {}
        # sources -> AllGather plan
        if k == 0:
            pool_pos, P, src_sched = None, 0, None
        else:
            srcs = np.unique(s_idx)
            per_core = [np.sort(srcs[(srcs // B) % NC == c]) for c in range(NC)]
            P = max(1, max(len(x) for x in per_core))
            pool_pos = {}
            swin_cb = set()
            slot_of = [dict() for _ in range(NC)]
            for c in range(NC):
                for slot, g in enumerate(per_core[c]):
                    g = int(g)
                    pool_pos[g] = c * P + slot
                    slot_of[c][g] = slot
                    lr = _local_row(g)
                    swin_cb.add((lr // B, (lr % B) // 128))
            swin_cb = sorted(swin_cb)
            sc_tables = []
            for (w, cb) in swin_cb:
                tab = np.full((NC, 128), OOB, dtype=np.int32)
                for c in range(NC):
                    blk = w * NC + c
                    if blk >= NBLKS:
                        continue
                    g0 = blk * B + cb * 128
                    for p in range(128):
                        s = slot_of[c].get(g0 + p)
                        if s is not None:
                            tab[c, p] = s
                sc_tables.append(tab)
            src_sched = {"swin_cb": swin_cb, "sc_tables": sc_tables}
        it["P"] = P
        it["src"] = src_sched

        # targets -> gather blocks + aggregation matrices
        tc = (t_idx // B) % NC
        tw = (t_idx // B) // NC
        hotwins = sorted(set(tw.tolist()))
        nblk_w = {}
        for w in hotwins:
            mx = 1
            for c in range(NC):
                ne = int(((tw == w) & (tc == c)).sum())
                mx = max(mx, (ne + 127) // 128)
            nblk_w[w] = mx
        nblk_total = sum(nblk_w.values())
        gidx = np.zeros((NC, nblk_total, 128), dtype=np.int32)
        smat = np.zeros((NC, nblk_total, 128, B), dtype=np.float32)
        bpos = 0
        blocks_of_w = {}
        for w in hotwins:
            blocks_of_w[w] = (bpos, nblk_w[w])
            for c in range(NC):
                m = (tw == w) & (tc == c)
                ss, tt = s_idx[m], t_idx[m]
                order = np.argsort(tt, kind="stable")
                ss, tt = ss[order], tt[order]
                for e in range(len(ss)):
                    b = bpos + e // 128
                    p = e % 128
                    gidx[c, b, p] = ss[e] if k == 0 else pool_pos[int(ss[e])]
                    smat[c, b, p, int(tt[e]) % B] = 1.0 / cnt[tt[e]]
            bpos += nblk_w[w]
        it["hotwins"] = hotwins
        it["blocks_of_w"] = blocks_of_w
        it["nblk_total"] = nblk_total
        it["gidx"] = gidx
        it["smat"] = smat
        its.append(it)
    return its


# ----------------------------------------------------------------------------
# bass program
# ----------------------------------------------------------------------------

def build_bass(sched):
    import concourse.bass as bass
    import concourse.bacc as bacc
    import concourse.mybir as mybir
    import concourse.tile as tile

    bf = mybir.dt.bfloat16
    f32 = mybir.dt.float32
    i32 = mybir.dt.int32
    AF = mybir.ActivationFunctionType
    Alu = mybir.AluOpType

    NGB = sum(it["nblk_total"] for it in sched)
    NSB = sum(len(it["src"]["swin_cb"]) for it in sched if it["src"]) or 1

    nc = bacc.Bacc("TRN2", target_bir_lowering=False, debug=False,
                   enable_asserts=True, num_devices=NC)
    VT0 = nc.dram_tensor("VT0", [WPC, 128, 3, B], bf, kind="ExternalInput").ap()
    VF = nc.dram_tensor("VF", [N, D], bf, kind="ExternalInput").ap()
    WCAT = nc.dram_tensor("WCAT", [128, 6, 9, 128], bf, kind="ExternalInput").ap()
    BLHS = nc.dram_tensor("BLHS", [1, 12, 128], bf, kind="ExternalInput").ap()
    BCOL = nc.dram_tensor("BCOL", [128, 12], bf, kind="ExternalInput").ap()
    IDN = nc.dram_tensor("IDN", [128, 128], bf, kind="ExternalInput").ap()
    GIDX = nc.dram_tensor("GIDX", [128, NGB], i32, kind="ExternalInput").ap()
    SIDX = nc.dram_tensor("SIDX", [128, NSB], i32, kind="ExternalInput").ap()
    SMAT = nc.dram_tensor("SMAT", [NGB, 128, B], bf, kind="ExternalInput").ap()
    OUT = nc.dram_tensor("OUT", [WPC, 128, 3, B], bf, kind="ExternalOutput").ap()

    with tile.TileContext(nc) as tc:
        with tc.tile_pool(name="const", bufs=1) as cp, \
             tc.tile_pool(name="state", bufs=1) as st, \
             tc.tile_pool(name="work", bufs=3) as wk, \
             tc.tile_pool(name="psum", bufs=2, space="PSUM") as ps, \
             tc.tile_pool(name="dram", bufs=1, space="DRAM") as dp:

            # resident constants
            wcat = cp.tile([128, 6, 9, 128], bf)
            nc.sync.dma_start(out=wcat[:], in_=WCAT[:])
            blhs = cp.tile([1, 12, 128], bf)
            nc.sync.dma_start(out=blhs[:], in_=BLHS[:])
            bcol = cp.tile([128, 12], bf)
            nc.sync.dma_start(out=bcol[:], in_=BCOL[:])
            idn = cp.tile([128, 128], bf)
            nc.sync.dma_start(out=idn[:], in_=IDN[:])
            gidx = cp.tile([128, NGB], i32)
            nc.sync.dma_start(out=gidx[:], in_=GIDX[:])
            sidx = cp.tile([128, NSB], i32)
            nc.sync.dma_start(out=sidx[:], in_=SIDX[:])
            ones = cp.tile([1, B], bf)
            nc.vector.memset(ones[:], 1.0)
            # b_ih_n broadcast [128, 3, 384] for cold windows
            binb = cp.tile([128, 3, B], bf)
            nc.vector.tensor_copy(binb[:], bcol[:, 9:12, None].to_broadcast([128, 3, B]))

            # state: double-buffered transposed hidden, per window
            hT = [[st.tile([128, 3, B], bf, tag=f"h{buf}w{w}", name=f"h{buf}w{w}")
                   for w in range(WPC)] for buf in range(2)]
            for w in range(WPC):
                nc.sync.dma_start(out=hT[0][w][:], in_=VT0[w])
            # per-window q tiles (only hot windows get written)
            qs = [st.tile([128, 3, B], bf, tag=f"q{w}", name=f"q{w}") for w in range(WPC)]

            gpos = 0
            spos = 0
            for k in range(ITERS):
                it = sched[k]
                cur, nxt = k % 2, (k + 1) % 2
                if k == 0:
                    src_ap = VF
                # phase A: gather + aggregate messages into qs[w]
                for w in it["hotwins"]:
                    bpos, nb = it["blocks_of_w"][w]
                    qp = ps.tile([128, 3, 512], f32, tag="g3", space="PSUM")
                    for bi in range(nb):
                        xg = wk.tile([128, D], bf, tag="xg")
                        nc.gpsimd.indirect_dma_start(
                            out=xg[:], out_offset=None, in_=src_ap[:],
                            in_offset=bass.IndirectOffsetOnAxis(
                                ap=gidx[:, gpos:gpos + 1], axis=0))
                        sm = wk.tile([128, B], bf, tag="sm")
                        nc.sync.dma_start(out=sm[:], in_=SMAT[gpos])
                        for kt in range(3):
                            nc.tensor.matmul(
                                qp[:, kt, :B],
                                lhsT=xg[:, kt * 128:(kt + 1) * 128],
                                rhs=sm[:],
                                start=(bi == 0), stop=(bi == nb - 1))
                        gpos += 1
                    nc.any.tensor_copy(qs[w][:], qp[:, :, :B])

                # phase B: dense GRU per window
                for w in range(WPC):
                    hot = w in it["hotwins"]
                    hcur = hT[cur][w]

                    def gate_psum(mlo, bias_slot, with_q, with_h=True):
                        gp = ps.tile([128, 3, 512], f32, tag="g3", space="PSUM")
                        for j in range(3):
                            m = mlo + j
                            out = gp[:, j, :B]
                            nc.tensor.matmul(out, lhsT=blhs[:, bias_slot + j, :],
                                             rhs=ones[:], start=True, stop=False)
                            if with_h:
                                for kt in range(3):
                                    nc.tensor.matmul(
                                        out, lhsT=wcat[:, 3 + kt, m, :],
                                        rhs=hcur[:, kt, :],
                                        start=False,
                                        stop=(not with_q and kt == 2))
                            if with_q:
                                for kt in range(3):
                                    nc.tensor.matmul(
                                        out, lhsT=wcat[:, kt, m, :],
                                        rhs=qs[w][:, kt, :],
                                        start=False, stop=(kt == 2))
                        return gp

                    rp = gate_psum(0, 0, hot)
                    r_sb = wk.tile([128, 3, B], bf, tag="r")
                    nc.scalar.activation(r_sb[:], rp[:, :, :B], AF.Sigmoid)
                    zp = gate_psum(3, 3, hot)
                    z_sb = wk.tile([128, 3, B], bf, tag="z")
                    nc.scalar.activation(z_sb[:], zp[:, :, :B], AF.Sigmoid)
                    hp = gate_psum(6, 6, False)
                    t1 = wk.tile([128, 3, B], bf, tag="t1")
                    nc.vector.tensor_tensor(out=t1[:], in0=r_sb[:], in1=hp[:, :, :B],
                                            op=Alu.mult)
                    t2 = wk.tile([128, 3, B], bf, tag="t2")
                    if hot:
                        ip = gate_psum(6, 9, True, with_h=False)
                        nc.vector.tensor_tensor(out=t2[:], in0=t1[:], in1=ip[:, :, :B],
                                                op=Alu.add)
                    else:
                        nc.vector.tensor_tensor(out=t2[:], in0=t1[:], in1=binb[:],
                                                op=Alu.add)
                    n_sb = wk.tile([128, 3, B], bf, tag="n")
                    nc.scalar.activation(n_sb[:], t2[:], AF.Tanh)
                    d_sb = wk.tile([128, 3, B], bf, tag="d")
                    nc.vector.tensor_sub(out=d_sb[:], in0=hcur[:], in1=n_sb[:])
                    e_sb = wk.tile([128, 3, B], bf, tag="e")
                    nc.vector.tensor_mul(out=e_sb[:], in0=z_sb[:], in1=d_sb[:])
                    nc.vector.tensor_add(out=hT[nxt][w][:], in0=n_sb[:], in1=e_sb[:])

                # phase C: extract next iteration's sources, AllGather
                if k + 1 < ITERS:
                    nx = sched[k + 1]
                    P = nx["P"]
                    agin = dp.tile([P, D], bf, tag=f"agin{k+1}")
                    agout = dp.tile([NC * P, D], bf, tag=f"agout{k+1}",
                                    addr_space="Shared")
                    for (w, cb) in nx["src"]["swin_cb"]:
                        tp = ps.tile([128, B], bf, tag="tp", space="PSUM")
                        for kt in range(3):
                            nc.tensor.transpose(
                                tp[:, kt * 128:(kt + 1) * 128],
                                hT[nxt][w][:, kt, cb * 128:(cb + 1) * 128],
                                idn[:])
                        rm = wk.tile([128, D], bf, tag="rm")
                        nc.any.tensor_copy(rm[:], tp[:])
                        nc.gpsimd.indirect_dma_start(
                            out=agin[:],
                            out_offset=bass.IndirectOffsetOnAxis(
                                ap=sidx[:, spos:spos + 1], axis=0),
                            in_=rm[:], in_offset=None,
                            bounds_check=P - 1, oob_is_err=False)
                        spos += 1
                    nc.gpsimd.collective_compute(
                        "AllGather", Alu.bypass,
                        replica_groups=[list(range(NC))],
                        ins=[agin[:].opt()], outs=[agout[:].opt()])
                    src_ap = agout

            last = ITERS % 2
            for w in range(WPC):
                nc.sync.dma_start(out=OUT[w], in_=hT[last][w][:])
    nc.compile()
    return nc


# ----------------------------------------------------------------------------
# host packing + entry point
# ----------------------------------------------------------------------------

def pack_inputs(sched, c, V, conv_weight, w_ih, w_hh, b_ih, b_hh):
    V = np.asarray(V, dtype=np.float32)
    Wcat = np.concatenate([np.asarray(conv_weight) @ np.asarray(w_ih).T,
                           np.asarray(w_hh).T], axis=0).astype(np.float32)
    b_ih = np.asarray(b_ih, dtype=np.float32)
    b_hh = np.asarray(b_hh, dtype=np.float32)

    grows = _global_rows_of(c)
    hl = np.zeros((LROWS, D), dtype=np.float32)
    valid = grows >= 0
    hl[valid] = V[grows[valid]]
    # VT0[w, p, kt, j] = h[w*B + j, kt*128 + p]
    vt0 = np.ascontiguousarray(
        hl.reshape(WPC, B, 3, 128).transpose(0, 3, 2, 1)).astype(BF16)
    # WCAT[p, kpart, m, :]: kpart 0-2 -> Wcat rows (ih), 3-5 -> rows 384+
    wc = np.zeros((128, 6, 9, 128), dtype=np.float32)
    for kp in range(6):
        for m in range(9):
            wc[:, kp, m, :] = Wcat[kp * 128:(kp + 1) * 128, m * 128:(m + 1) * 128]
    bsum = b_ih + b_hh
    bl = np.zeros((1, 12, 128), dtype=np.float32)
    for m in range(6):
        bl[0, m] = bsum[m * 128:(m + 1) * 128]
    for j in range(3):
        bl[0, 6 + j] = b_hh[768 + j * 128: 768 + (j + 1) * 128]
        bl[0, 9 + j] = b_ih[768 + j * 128: 768 + (j + 1) * 128]
    bc = np.ascontiguousarray(bl[0].T)  # [128, 12]

    gidx = np.concatenate([it["gidx"][c] for it in sched], axis=0)  # [NGB,128]
    smat = np.concatenate([it["smat"][c] for it in sched], axis=0)  # [NGB,128,B]
    sc = [tab[c] for it in sched if it["src"] for tab in it["src"]["sc_tables"]]
    sidx = (np.stack(sc, axis=0) if sc else np.zeros((1, 128), np.int32))

    return {
        "VT0": vt0,
        "VF": V.astype(BF16),
        "WCAT": wc.astype(BF16),
        "BLHS": bl.astype(BF16),
        "BCOL": bc.astype(BF16),
        "IDN": np.eye(128, dtype=np.float32).astype(BF16),
        "GIDX": np.ascontiguousarray(gidx.T).astype(np.int32),
        "SIDX": np.ascontiguousarray(sidx.T).astype(np.int32),
        "SMAT": smat.astype(BF16),
    }


def unpack_output(results):
    out = np.zeros((N, D), dtype=np.float32)
    for c in range(NC):
        o = np.asarray(results[c]["OUT"], dtype=np.float32)  # [WPC,128,3,B]
        hl = o.transpose(0, 3, 2, 1).reshape(LROWS, D)
        grows = _global_rows_of(c)
        valid = grows >= 0
        out[grows[valid]] = hl[valid]
    return out


_CACHE = {}


def _install_profile_hook():
    """The agent image lacks ``antenv.axon_hooks``; shim it so
    run_bass_kernel_spmd(trace=True) can capture NTFF profiles."""
    import types
    try:
        from antenv.axon_hooks import get_axon_ntff_profile_hook  # noqa: F401
        return True
    except ImportError:
        pass
    try:
        import antenv
        from trn_agent_boot.trn_boot import _ntff_profile_via_ctypes
        hook = _ntff_profile_via_ctypes("/opt/axon/libaxon_pjrt.so")
        mod = types.ModuleType("antenv.axon_hooks")
        mod._hook = hook
        mod.set_axon_ntff_profile_hook = lambda h: setattr(mod, "_hook", h)
        mod.get_axon_ntff_profile_hook = lambda: mod._hook
        sys.modules["antenv.axon_hooks"] = mod
        antenv.axon_hooks = mod
        return hook is not None
    except Exception:
        return False


def kernel(V, E, conv_weight, w_ih, w_hh, b_ih, b_hh, _want_results=False):
    from concourse import bass_utils
    E_np = np.asarray(E)
    key = ("prog",)
    sched = build_schedule(E_np)
    if key not in _CACHE:
        _CACHE[key] = build_bass(sched)
    nc = _CACHE[key]
    in_maps = [pack_inputs(sched, c, V, conv_weight, w_ih, w_hh, b_ih, b_hh)
               for c in range(NC)]
    trace = os.environ.get("KERNEL_TRACE", "0") == "1"
    if trace:
        trace = _install_profile_hook()
        # artifact upload to the fish bucket is unavailable here; stub it
        bass_utils.upload_artifacts = lambda tmpdir: "local://" + str(tmpdir)
    res = bass_utils.run_bass_kernel_spmd(
        nc, in_maps, core_ids=list(range(NC)), trace=trace,
        tmpdir=os.environ.get("KERNEL_TMPDIR"))
    out = unpack_output(res.results).astype(np.float32)
    if _want_results:
        return out, res
    return out


# revision 7
# speedup vs baseline: 1.5440x; 1.5440x over previous
"""Trainium2 Bass kernel for nn_AST_LSTM (GRU-based AST message passing).

Algorithm notes
---------------
The reference peels leaf edges of a random tree for 15 iterations; the
edge schedule (which edges fire when, and the compacted index remapping)
depends ONLY on E, so it is precomputed on the host. Per iteration the
device does, for each of 8 row-sharded cores:

    q = S_k @ h            (sparse mean-aggregate of gathered rows)
    G = [q | h] @ Wcat + b  with Wcat = [conv_w @ w_ih.T ; w_hh.T]
    r = sigmoid(G[:, :384]); z = sigmoid(G[:, 384:768])
    n = tanh(gi_n + b_ih_n + r * (gh_n + b_hh_n))
    h' = n + z * (h - n)

Rows are block-cyclically sharded (block=384) over 8 cores; each core
holds h transposed in SBUF as 10 window tiles [128, 3, 384] (feature
partition-major) so all matmuls run directly.  Message sources are
exchanged once per iteration with an AllGather of just the needed rows
(the "halo"); iteration 0 gathers from a full local copy of V.  All
device data is bf16 (validated: final rel err ~1e-3); PSUM accumulates
fp32; biases are injected into PSUM via K=1 ones-matmuls.
"""
import os
import sys
import numpy as np

sys.path.insert(0, "/opt/trn_rl_repo")
import ml_dtypes

N = 30000
D = 384
NC = 8
ITERS = int(os.environ.get("KERNEL_ITERS", "15"))
B = 384              # assignment block == window width
WPC = 10             # windows per core
LROWS = B * WPC      # 3840 local rows (padded)
NBLKS = (N + B - 1) // B
OOB = 1 << 20
BF16 = ml_dtypes.bfloat16

# ----------------------------------------------------------------------------
# host-side schedule
# ----------------------------------------------------------------------------

def _local_row(g):
    return ((g // B) // NC) * B + g % B


def _global_rows_of(c):
    out = np.full(LROWS, -1, dtype=np.int64)
    for w in range(WPC):
        blk = w * NC + c
        if blk >= NBLKS:
            continue
        g0 = blk * B
        n = min(B, N - g0)
        out[w * B: w * B + n] = np.arange(g0, g0 + n)
    return out


def _peel_schedule(E):
    src = np.asarray(E[0], dtype=np.int64)
    dst = np.asarray(E[1], dtype=np.int64)
    M = src.shape[0]
    active = np.ones(M, dtype=bool)
    iters = []
    for _ in range(ITERS):
        tgt_cnt = np.zeros(N, np.int64)
        np.add.at(tgt_cnt, dst, active.astype(np.int64))
        use = active & (tgt_cnt == 0)[src]
        ui = use.astype(np.int64)
        node_used = np.zeros(N, np.int64)
        np.maximum.at(node_used, src, ui)
        np.maximum.at(node_used, dst, ui)
        index_map = np.cumsum(node_used) - 1
        s_idx = index_map[src[use]]
        t_idx = index_map[dst[use]]
        cnt = np.zeros(N, np.int64)
        np.add.at(cnt, t_idx, 1)
        iters.append((s_idx, t_idx, cnt))
        active = active & ~use
    return iters


def build_schedule(E):
    """Static schedule: identical program structure for all cores, per-core
    index/matrix data (padded to union shapes)."""
    peel = _peel_schedule(E)
    its = []
    for k in range(ITERS):
        s_idx, t_idx, cnt = peel[k]
        it = {}
        # sources -> AllGather plan
        if k == 0:
            pool_pos, P, src_sched = None, 0, None
        else:
            srcs = np.unique(s_idx)
            per_core = [np.sort(srcs[(srcs // B) % NC == c]) for c in range(NC)]
            P = max(1, max(len(x) for x in per_core))
            pool_pos = {}
            swin_cb = set()
            slot_of = [dict() for _ in range(NC)]
            for c in range(NC):
                for slot, g in enumerate(per_core[c]):
                    g = int(g)
                    pool_pos[g] = c * P + slot
                    slot_of[c][g] = slot
                    lr = _local_row(g)
                    swin_cb.add((lr // B, (lr % B) // 128))
            swin_cb = sorted(swin_cb)
            sc_tables = []
            for (w, cb) in swin_cb:
                tab = np.full((NC, 128), OOB, dtype=np.int32)
                for c in range(NC):
                    blk = w * NC + c
                    if blk >= NBLKS:
                        continue
                    g0 = blk * B + cb * 128
                    for p in range(128):
                        s = slot_of[c].get(g0 + p)
                        if s is not None:
                            tab[c, p] = s
                sc_tables.append(tab)
            src_sched = {"swin_cb": swin_cb, "sc_tables": sc_tables}
        it["P"] = P
        it["src"] = src_sched

        # targets -> gather blocks + aggregation matrices
        tc = (t_idx // B) % NC
        tw = (t_idx // B) // NC
        hotwins = sorted(set(tw.tolist()))
        nblk_w = {}
        for w in hotwins:
            mx = 1
            for c in range(NC):
                ne = int(((tw == w) & (tc == c)).sum())
                mx = max(mx, (ne + 127) // 128)
            nblk_w[w] = mx
        nblk_total = sum(nblk_w.values())
        gidx = np.zeros((NC, nblk_total, 128), dtype=np.int32)
        smat = np.zeros((NC, nblk_total, 128, B), dtype=np.float32)
        bpos = 0
        blocks_of_w = {}
        for w in hotwins:
            blocks_of_w[w] = (bpos, nblk_w[w])
            for c in range(NC):
                m = (tw == w) & (tc == c)
                ss, tt = s_idx[m], t_idx[m]
                order = np.argsort(tt, kind="stable")
                ss, tt = ss[order], tt[order]
                for e in range(len(ss)):
                    b = bpos + e // 128
                    p = e % 128
                    gidx[c, b, p] = ss[e] if k == 0 else pool_pos[int(ss[e])]
                    smat[c, b, p, int(tt[e]) % B] = 1.0 / cnt[tt[e]]
            bpos += nblk_w[w]
        it["hotwins"] = hotwins
        it["blocks_of_w"] = blocks_of_w
        it["nblk_total"] = nblk_total
        it["gidx"] = gidx
        it["smat"] = smat
        its.append(it)
    return its


# ----------------------------------------------------------------------------
# bass program
# ----------------------------------------------------------------------------

def build_bass(sched):
    import concourse.bass as bass
    import concourse.bacc as bacc
    import concourse.mybir as mybir
    import concourse.tile as tile

    bf = mybir.dt.bfloat16
    f32 = mybir.dt.float32
    i32 = mybir.dt.int32
    AF = mybir.ActivationFunctionType
    Alu = mybir.AluOpType

    NGB = sum(it["nblk_total"] for it in sched)
    NSB = sum(len(it["src"]["swin_cb"]) for it in sched if it["src"]) or 1

    nc = bacc.Bacc("TRN2", target_bir_lowering=False, debug=False,
                   enable_asserts=True, num_devices=NC)
    VT0 = nc.dram_tensor("VT0", [WPC, 128, 3, B], bf, kind="ExternalInput").ap()
    VF = nc.dram_tensor("VF", [N, D], bf, kind="ExternalInput").ap()
    WCAT = nc.dram_tensor("WCAT", [128, 6, 9, 128], bf, kind="ExternalInput").ap()
    BLHS = nc.dram_tensor("BLHS", [1, 12, 128], bf, kind="ExternalInput").ap()
    BCOL = nc.dram_tensor("BCOL", [128, 12], bf, kind="ExternalInput").ap()
    IDN = nc.dram_tensor("IDN", [128, 128], bf, kind="ExternalInput").ap()
    GIDX = nc.dram_tensor("GIDX", [128, NGB], i32, kind="ExternalInput").ap()
    SIDX = nc.dram_tensor("SIDX", [128, NSB], i32, kind="ExternalInput").ap()
    SMAT = nc.dram_tensor("SMAT", [NGB, 128, B], bf, kind="ExternalInput").ap()
    OUT = nc.dram_tensor("OUT", [WPC, 128, 3, B], bf, kind="ExternalOutput").ap()

    with tile.TileContext(nc) as tc:
        with tc.tile_pool(name="const", bufs=1) as cp, \
             tc.tile_pool(name="state", bufs=1) as st, \
             tc.tile_pool(name="work", bufs=3) as wk, \
             tc.tile_pool(name="psum", bufs=2, space="PSUM") as ps, \
             tc.tile_pool(name="dram", bufs=1, space="DRAM") as dp:

            # resident constants
            wcat = cp.tile([128, 6, 9, 128], bf)
            nc.sync.dma_start(out=wcat[:], in_=WCAT[:])
            blhs = cp.tile([1, 12, 128], bf)
            nc.sync.dma_start(out=blhs[:], in_=BLHS[:])
            bcol = cp.tile([128, 12], bf)
            nc.sync.dma_start(out=bcol[:], in_=BCOL[:])
            idn = cp.tile([128, 128], bf)
            nc.sync.dma_start(out=idn[:], in_=IDN[:])
            gidx = cp.tile([128, NGB], i32)
            nc.sync.dma_start(out=gidx[:], in_=GIDX[:])
            sidx = cp.tile([128, NSB], i32)
            nc.sync.dma_start(out=sidx[:], in_=SIDX[:])
            ones = cp.tile([1, B], bf)
            nc.vector.memset(ones[:], 1.0)
            # b_ih_n broadcast [128, 3, 384] for cold windows
            binb = cp.tile([128, 3, B], bf)
            nc.vector.tensor_copy(binb[:], bcol[:, 9:12, None].to_broadcast([128, 3, B]))

            # state: double-buffered transposed hidden, per window
            hT = [[st.tile([128, 3, B], bf, tag=f"h{buf}w{w}", name=f"h{buf}w{w}")
                   for w in range(WPC)] for buf in range(2)]
            for w in range(WPC):
                nc.sync.dma_start(out=hT[0][w][:], in_=VT0[w])
            # per-window q tiles (only hot windows get written)
            qs = [st.tile([128, 3, B], bf, tag=f"q{w}", name=f"q{w}") for w in range(WPC)]

            gpos = 0
            spos = 0
            for k in range(ITERS):
                it = sched[k]
                cur, nxt = k % 2, (k + 1) % 2
                if k == 0:
                    src_ap = VF
                # per-window phase C info for iteration k+1 sources
                src_cbs = {}
                if k + 1 < ITERS:
                    for (w, cb) in sched[k + 1]["src"]["swin_cb"]:
                        src_cbs.setdefault(w, []).append(cb)
                    P1 = sched[k + 1]["P"]
                    agin = dp.tile([P1, D], bf, tag=f"agin{k+1}",
                                   name=f"agin{k+1}")
                    agout = dp.tile([NC * P1, D], bf, tag=f"agout{k+1}",
                                    name=f"agout{k+1}", addr_space="Shared")

                # phase A: gather + aggregate messages into qs[w]
                for w in it["hotwins"]:
                    bpos, nb = it["blocks_of_w"][w]
                    qp = ps.tile([128, 3, 512], f32, tag="g3", space="PSUM")
                    for bi in range(nb):
                        xg = wk.tile([128, D], bf, tag="xg")
                        nc.gpsimd.indirect_dma_start(
                            out=xg[:], out_offset=None, in_=src_ap[:],
                            in_offset=bass.IndirectOffsetOnAxis(
                                ap=gidx[:, gpos:gpos + 1], axis=0))
                        sm = wk.tile([128, B], bf, tag="sm")
                        nc.sync.dma_start(out=sm[:], in_=SMAT[gpos])
                        for kt in range(3):
                            nc.tensor.matmul(
                                qp[:, kt, :B],
                                lhsT=xg[:, kt * 128:(kt + 1) * 128],
                                rhs=sm[:],
                                start=(bi == 0), stop=(bi == nb - 1))
                        gpos += 1
                    nc.vector.tensor_copy(qs[w][:], qp[:, :, :B])

                # phase B: source/hot windows first so the AllGather for the
                # next iteration launches early; cold windows after it keep
                # the PE busy while the collective + gathers run.
                early = sorted(set(it["hotwins"]) | set(src_cbs))
                worder = early + [w for w in range(WPC) if w not in early]
                for w in worder:
                    hot = w in it["hotwins"]
                    hcur = hT[cur][w]

                    def gate_psum(mlo, bias_slot, with_q, with_h=True):
                        gp = ps.tile([128, 3, 512], f32, tag="g3", space="PSUM")
                        for j in range(3):
                            m = mlo + j
                            out = gp[:, j, :B]
                            nc.tensor.matmul(out, lhsT=blhs[:, bias_slot + j, :],
                                             rhs=ones[:], start=True, stop=False)
                            if with_h:
                                for kt in range(3):
                                    nc.tensor.matmul(
                                        out, lhsT=wcat[:, 3 + kt, m, :],
                                        rhs=hcur[:, kt, :],
                                        start=False,
                                        stop=(not with_q and kt == 2))
                            if with_q:
                                for kt in range(3):
                                    nc.tensor.matmul(
                                        out, lhsT=wcat[:, kt, m, :],
                                        rhs=qs[w][:, kt, :],
                                        start=False, stop=(kt == 2))
                        return gp

                    rp = gate_psum(0, 0, hot)
                    r_sb = wk.tile([128, 3, B], bf, tag="r")
                    nc.scalar.activation(r_sb[:], rp[:, :, :B], AF.Sigmoid)
                    zp = gate_psum(3, 3, hot)
                    z_sb = wk.tile([128, 3, B], bf, tag="z")
                    nc.scalar.activation(z_sb[:], zp[:, :, :B], AF.Sigmoid)
                    hp = gate_psum(6, 6, False)
                    t1 = wk.tile([128, 3, B], bf, tag="t1")
                    nc.vector.tensor_tensor(out=t1[:], in0=r_sb[:], in1=hp[:, :, :B],
                                            op=Alu.mult)
                    t2 = wk.tile([128, 3, B], bf, tag="t2")
                    if hot:
                        ip = gate_psum(6, 9, True, with_h=False)
                        nc.vector.tensor_tensor(out=t2[:], in0=t1[:], in1=ip[:, :, :B],
                                                op=Alu.add)
                    else:
                        nc.vector.tensor_tensor(out=t2[:], in0=t1[:], in1=binb[:],
                                                op=Alu.add)
                    n_sb = wk.tile([128, 3, B], bf, tag="n")
                    nc.scalar.activation(n_sb[:], t2[:], AF.Tanh)
                    d_sb = wk.tile([128, 3, B], bf, tag="d")
                    nc.vector.tensor_sub(out=d_sb[:], in0=hcur[:], in1=n_sb[:])
                    e_sb = wk.tile([128, 3, B], bf, tag="e")
                    nc.vector.tensor_mul(out=e_sb[:], in0=z_sb[:], in1=d_sb[:])
                    nc.vector.tensor_add(out=hT[nxt][w][:], in0=n_sb[:], in1=e_sb[:])

                    # phase C (inline): extract next-iteration sources from
                    # this window as soon as its h' is ready
                    for cb in src_cbs.get(w, []):
                        tp = ps.tile([128, B], bf, tag="tp", space="PSUM")
                        for kt in range(3):
                            nc.tensor.transpose(
                                tp[:, kt * 128:(kt + 1) * 128],
                                hT[nxt][w][:, kt, cb * 128:(cb + 1) * 128],
                                idn[:])
                        rm = wk.tile([128, D], bf, tag="rm")
                        nc.vector.tensor_copy(rm[:], tp[:])
                        nc.gpsimd.indirect_dma_start(
                            out=agin[:],
                            out_offset=bass.IndirectOffsetOnAxis(
                                ap=sidx[:, spos:spos + 1], axis=0),
                            in_=rm[:], in_offset=None,
                            bounds_check=P1 - 1, oob_is_err=False)
                        spos += 1
                    if k + 1 < ITERS and w == max(early):
                        nc.gpsimd.collective_compute(
                            "AllGather", Alu.bypass,
                            replica_groups=[list(range(NC))],
                            ins=[agin[:].opt()], outs=[agout[:].opt()])
                if k + 1 < ITERS:
                    src_ap = agout

            last = ITERS % 2
            for w in range(WPC):
                nc.sync.dma_start(out=OUT[w], in_=hT[last][w][:])
    nc.compile()
    return nc


# ----------------------------------------------------------------------------
# host packing + entry point
# ----------------------------------------------------------------------------

def pack_inputs(sched, c, V, conv_weight, w_ih, w_hh, b_ih, b_hh):
    V = np.asarray(V, dtype=np.float32)
    Wcat = np.concatenate([np.asarray(conv_weight) @ np.asarray(w_ih).T,
                           np.asarray(w_hh).T], axis=0).astype(np.float32)
    b_ih = np.asarray(b_ih, dtype=np.float32)
    b_hh = np.asarray(b_hh, dtype=np.float32)

    grows = _global_rows_of(c)
    hl = np.zeros((LROWS, D), dtype=np.float32)
    valid = grows >= 0
    hl[valid] = V[grows[valid]]
    # VT0[w, p, kt, j] = h[w*B + j, kt*128 + p]
    vt0 = np.ascontiguousarray(
        hl.reshape(WPC, B, 3, 128).transpose(0, 3, 2, 1)).astype(BF16)
    # WCAT[p, kpart, m, :]: kpart 0-2 -> Wcat rows (ih), 3-5 -> rows 384+
    wc = np.zeros((128, 6, 9, 128), dtype=np.float32)
    for kp in range(6):
        for m in range(9):
            wc[:, kp, m, :] = Wcat[kp * 128:(kp + 1) * 128, m * 128:(m + 1) * 128]
    bsum = b_ih + b_hh
    bl = np.zeros((1, 12, 128), dtype=np.float32)
    for m in range(6):
        bl[0, m] = bsum[m * 128:(m + 1) * 128]
    for j in range(3):
        bl[0, 6 + j] = b_hh[768 + j * 128: 768 + (j + 1) * 128]
        bl[0, 9 + j] = b_ih[768 + j * 128: 768 + (j + 1) * 128]
    bc = np.ascontiguousarray(bl[0].T)  # [128, 12]

    gidx = np.concatenate([it["gidx"][c] for it in sched], axis=0)  # [NGB,128]
    smat = np.concatenate([it["smat"][c] for it in sched], axis=0)  # [NGB,128,B]
    sc = [tab[c] for it in sched if it["src"] for tab in it["src"]["sc_tables"]]
    sidx = (np.stack(sc, axis=0) if sc else np.zeros((1, 128), np.int32))

    return {
        "VT0": vt0,
        "VF": V.astype(BF16),
        "WCAT": wc.astype(BF16),
        "BLHS": bl.astype(BF16),
        "BCOL": bc.astype(BF16),
        "IDN": np.eye(128, dtype=np.float32).astype(BF16),
        "GIDX": np.ascontiguousarray(gidx.T).astype(np.int32),
        "SIDX": np.ascontiguousarray(sidx.T).astype(np.int32),
        "SMAT": smat.astype(BF16),
    }


def unpack_output(results):
    out = np.zeros((N, D), dtype=np.float32)
    for c in range(NC):
        o = np.asarray(results[c]["OUT"], dtype=np.float32)  # [WPC,128,3,B]
        hl = o.transpose(0, 3, 2, 1).reshape(LROWS, D)
        grows = _global_rows_of(c)
        valid = grows >= 0
        out[grows[valid]] = hl[valid]
    return out


_CACHE = {}


def _install_profile_hook():
    """The agent image lacks ``antenv.axon_hooks``; shim it so
    run_bass_kernel_spmd(trace=True) can capture NTFF profiles."""
    import types
    try:
        from antenv.axon_hooks import get_axon_ntff_profile_hook  # noqa: F401
        return True
    except ImportError:
        pass
    try:
        import antenv
        from trn_agent_boot.trn_boot import _ntff_profile_via_ctypes
        hook = _ntff_profile_via_ctypes("/opt/axon/libaxon_pjrt.so")
        mod = types.ModuleType("antenv.axon_hooks")
        mod._hook = hook
        mod.set_axon_ntff_profile_hook = lambda h: setattr(mod, "_hook", h)
        mod.get_axon_ntff_profile_hook = lambda: mod._hook
        sys.modules["antenv.axon_hooks"] = mod
        antenv.axon_hooks = mod
        return hook is not None
    except Exception:
        return False


def kernel(V, E, conv_weight, w_ih, w_hh, b_ih, b_hh, _want_results=False):
    from concourse import bass_utils
    E_np = np.asarray(E)
    key = ("prog",)
    sched = build_schedule(E_np)
    if key not in _CACHE:
        _CACHE[key] = build_bass(sched)
    nc = _CACHE[key]
    in_maps = [pack_inputs(sched, c, V, conv_weight, w_ih, w_hh, b_ih, b_hh)
               for c in range(NC)]
    trace = os.environ.get("KERNEL_TRACE", "0") == "1"
    if trace:
        trace = _install_profile_hook()
        # artifact upload to the fish bucket is unavailable here; stub it
        bass_utils.upload_artifacts = lambda tmpdir: "local://" + str(tmpdir)
    res = bass_utils.run_bass_kernel_spmd(
        nc, in_maps, core_ids=list(range(NC)), trace=trace,
        tmpdir=os.environ.get("KERNEL_TMPDIR"))
    out = unpack_output(res.results).astype(np.float32)
    if _want_results:
        return out, res
    return out
